# revision 1
# baseline (speedup 1.0000x reference)
"""DMSTGCN forward on 8 Trainium2 NeuronCores (Bass/Tile).

Self-contained: hardcodes all shapes. kernel(**inputs) takes the full
(unsharded) numpy inputs and returns the full [64, 3, 500, 1] output.

Sharding: data-parallel over batch B=64 -> 8 samples per core. Batchnorm
statistics are AllReduced across cores each layer. Adjacency matrices are
built on-device once per (group, sample), stored transposed in DRAM as
bf16, and streamed back per layer. All matmuls run in bf16 with fp32 PSUM
accumulation; batchnorm is folded into the next layer's weights on device.
"""
import os
import sys
from contextlib import ExitStack

import numpy as np

sys.path.insert(0, "/opt/trn_rl_repo")
os.environ.setdefault("JAX_PLATFORMS", "axon,cpu")

import ml_dtypes  # noqa: E402

# ---------------- static model constants ----------------
B, N, T = 64, 500, 12
RC, SC, EC, OUT = 16, 16 // 1, 16, 3  # SC fixed below
SC = 8
DIMS = 40
DILS = [1, 2, 4, 8]
RF = 16
T_INS = [16, 15, 13, 9]
T_OUTS = [15, 13, 9, 1]
CT_IN = [16, 240, 208, 144]    # (c,t) rows of layer input (l0: 1ch * 16t)
CT_OUT = [240, 208, 144, 16]
SCT = [SC * t for t in T_OUTS]  # 120, 104, 72, 8
SKIP_OFF = {3: 0, 2: 8, 1: 80, 0: 184}
EPS = 1e-5
NCORES = 8
BL = B // NCORES
V_TILES = [(0, 125), (125, 125), (250, 125), (375, 125)]


def pt_tiles(n):
    return [(o, min(128, n - o)) for o in range(0, n, 128)]


# ---------------- const packing registry (static shapes) ----------------
class Registry:
    def __init__(self):
        self.entries = {}
        self.size = 0

    def add(self, name, shape):
        n = int(np.prod(shape))
        self.entries[name] = (self.size, tuple(shape))
        self.size += n

    def off(self, name):
        return self.entries[name]


def build_registries():
    wreg = Registry()  # bf16 matmul constants
    breg = Registry()  # f32 bias/scalar constants
    for l in range(4):
        for s in range(4):
            wreg.add(f"Wf_{l}_{s}", (CT_IN[l], CT_OUT[l]))
            wreg.add(f"Wg_{l}_{s}", (CT_IN[l], CT_OUT[l]))
            breg.add(f"bf_{l}_{s}", (CT_OUT[l],))
            breg.add(f"bg_{l}_{s}", (CT_OUT[l],))
        if l == 0:
            for s in range(4):
                wreg.add(f"Rs0_{s}", (16, 240))
        else:
            wreg.add(f"Rsel_{l}", (CT_IN[l], CT_OUT[l]))
        for g in range(7):
            for m in range(3):
                wreg.add(f"G_{g}_{l}_{m}", (CT_OUT[l], CT_OUT[l]))
        wreg.add(f"Sk_{l}", (CT_OUT[l], SCT[l]))
        wreg.add(f"Exp_{l}", (16, RC * T_OUTS[l]))
        breg.add(f"gcbp_{l}", (CT_OUT[l],))
        for j in (1, 2, 3):
            breg.add(f"gcba_{l}_{j}", (CT_OUT[l],))
        breg.add(f"bng_{l}", (16, 4))
        breg.add(f"bnb_{l}", (16, 4))
    for s in range(4):
        breg.add(f"bres0_{s}", (240,))  # start_b expanded at T=15
    wreg.add("end1_lhsT", (304, EC))
    wreg.add("end2_lhsT", (EC, OUT))
    breg.add("skb", (304,))
    breg.add("end1_b", (EC,))
    breg.add("end2_b", (OUT,))
    return wreg, breg


WREG, BREG = build_registries()


# ---------------- host-side constant construction ----------------
def _banded(W2tap, d, T_in, T_out):
    O, C, _ = W2tap.shape
    M = np.zeros((C * T_in, O * T_out), np.float32)
    for o in range(O):
        for c in range(C):
            for to in range(T_out):
                M[c * T_in + to, o * T_out + to] += W2tap[o, c, 0]
                M[c * T_in + to + d, o * T_out + to] += W2tap[o, c, 1]
    return M


def _blockdiag(Wm, T_):
    O, C = Wm.shape
    M = np.zeros((C * T_, O * T_), np.float32)
    for o in range(O):
        for c in range(C):
            idx = np.arange(T_)
            M[c * T_ + idx, o * T_ + idx] = Wm[o, c]
    return M


def _residual_sel(T_in, T_out, C):
    off = T_in - T_out
    M = np.zeros((C * T_in, C * T_out), np.float32)
    for c in range(C):
        idx = np.arange(T_out)
        M[c * T_in + idx + off, c * T_out + idx] = 1.0
    return M


def _expand(vec, T_):
    return np.repeat(np.asarray(vec, np.float32), T_)


def host_constants(inputs):
    f32 = np.float32
    filt_W = np.asarray(inputs["filt_W"], f32); filt_b = np.asarray(inputs["filt_b"], f32)
    gate_W = np.asarray(inputs["gate_W"], f32); gate_b = np.asarray(inputs["gate_b"], f32)
    skip_W = np.asarray(inputs["skip_W"], f32); skip_b = np.asarray(inputs["skip_b"], f32)
    gc_W = np.asarray(inputs["gc_W"], f32); gc_b = np.asarray(inputs["gc_b"], f32)
    bn_g = np.asarray(inputs["bn_g"], f32); bn_b = np.asarray(inputs["bn_b"], f32)
    start_W = np.asarray(inputs["start_W"], f32); start_b = np.asarray(inputs["start_b"], f32)

    wc = np.zeros(WREG.size, f32)
    bc = np.zeros(BREG.size, f32)

    def wput(name, arr):
        off, shape = WREG.off(name)
        assert tuple(arr.shape) == shape, (name, arr.shape, shape)
        wc[off:off + arr.size] = arr.reshape(-1)

    def bput(name, arr):
        off, shape = BREG.off(name)
        assert tuple(arr.shape) == shape, (name, arr.shape, shape)
        bc[off:off + arr.size] = arr.reshape(-1)

    for l, d in enumerate(DILS):
        for s in range(4):
            if l == 0:
                sW = start_W[s][:, 0]
                fW = np.einsum("oct,c->ot", filt_W[s, 0], sW)[:, None, :]
                gW = np.einsum("oct,c->ot", gate_W[s, 0], sW)[:, None, :]
                wput(f"Wf_{l}_{s}", _banded(fW, d, 16, 15))
                wput(f"Wg_{l}_{s}", _banded(gW, d, 16, 15))
                bput(f"bf_{l}_{s}", _expand(filt_b[s, 0] + filt_W[s, 0].sum(-1) @ start_b[s], 15))
                bput(f"bg_{l}_{s}", _expand(gate_b[s, 0] + gate_W[s, 0].sum(-1) @ start_b[s], 15))
                M = np.zeros((16, RC * 15), f32)
                for c in range(RC):
                    idx = np.arange(15)
                    M[idx + 1, c * 15 + idx] = start_W[s][c, 0]
                wput(f"Rs0_{s}", M)
            else:
                wput(f"Wf_{l}_{s}", _banded(filt_W[s, l], d, T_INS[l], T_OUTS[l]))
                wput(f"Wg_{l}_{s}", _banded(gate_W[s, l], d, T_INS[l], T_OUTS[l]))
                bput(f"bf_{l}_{s}", _expand(filt_b[s, l], T_OUTS[l]))
                bput(f"bg_{l}_{s}", _expand(gate_b[s, l], T_OUTS[l]))
        if l > 0:
            wput(f"Rsel_{l}", _residual_sel(T_INS[l], T_OUTS[l], RC))
        for g in range(7):
            for m in range(3):
                wput(f"G_{g}_{l}_{m}", _blockdiag(gc_W[g, l][:, m * RC:(m + 1) * RC], T_OUTS[l]))
        wput(f"Sk_{l}", _blockdiag(skip_W[l], T_OUTS[l]))
        E = np.zeros((16, RC * T_OUTS[l]), f32)
        for c in range(RC):
            E[c, c * T_OUTS[l]:(c + 1) * T_OUTS[l]] = 1.0
        wput(f"Exp_{l}", E)
        bput(f"gcbp_{l}", _expand(gc_b[0, l] + gc_b[4, l] + gc_b[5, l] + gc_b[6, l], T_OUTS[l]))
        for j in (1, 2, 3):
            bput(f"gcba_{l}_{j}", _expand(gc_b[j, l], T_OUTS[l]))
        bput(f"bng_{l}", bn_g[:, l, :].T.copy())   # [16 (c), 4 (s)]
        bput(f"bnb_{l}", bn_b[:, l, :].T.copy())
    for s in range(4):
        bput(f"bres0_{s}", _expand(start_b[s], 15))
    wput("end1_lhsT", np.asarray(inputs["end1_W"], f32).T.copy())
    wput("end2_lhsT", np.asarray(inputs["end2_W"], f32).T.copy())
    skb = np.zeros(304, f32)
    for l in range(4):
        skb[SKIP_OFF[l]:SKIP_OFF[l] + SCT[l]] = _expand(skip_b[l], T_OUTS[l])
    bput("skb", skb)
    bput("end1_b", np.asarray(inputs["end1_b"], f32))
    bput("end2_b", np.asarray(inputs["end2_b"], f32))
    return wc.astype(ml_dtypes.bfloat16), bc


def host_per_core(inputs):
    """Per-core data tensors: xo [BL,4,16,500] bf16, t1 [7,BL,40,40] bf16."""
    f32 = np.float32
    x0 = np.asarray(inputs["x0"], f32)
    ind = np.asarray(inputs["ind"]).astype(np.int64)
    emb_t = np.asarray(inputs["emb_t"], f32)
    core = np.asarray(inputs["core"], f32)
    te = emb_t[:, ind, :]
    t1 = np.einsum("gbi,gijk->gbjk", te, core).astype(f32)
    xo = np.pad(x0, ((0, 0), (0, 0), (0, 0), (RF - T, 0)))
    xo = np.ascontiguousarray(np.transpose(xo, (0, 1, 3, 2)))
    se_T = np.ascontiguousarray(np.transpose(np.asarray(inputs["emb_s"], f32), (0, 2, 1)))
    de_T = np.ascontiguousarray(np.transpose(np.asarray(inputs["emb_d"], f32), (0, 2, 1)))
    bf = ml_dtypes.bfloat16
    return (xo.astype(bf), t1.astype(bf), se_T.astype(bf), de_T.astype(bf))


# ---------------- device program ----------------
_NC_CACHE = {}


def build_program(bl=BL, ncores=NCORES):
    import concourse.bacc as bacc
    import concourse.bass as bass
    import concourse.tile as tile
    import concourse.mybir as mybir
    from concourse import masks

    f32 = mybir.dt.float32
    bf = mybir.dt.bfloat16
    AF = mybir.ActivationFunctionType
    ALU = mybir.AluOpType

    nc = bacc.Bacc("TRN2", target_bir_lowering=False, debug=False,
                   num_devices=ncores)

    xo_d = nc.dram_tensor("xo", [bl, 4, 16, N], bf, kind="ExternalInput")
    t1_d = nc.dram_tensor("t1", [7, bl, DIMS, DIMS], bf, kind="ExternalInput")
    seT_d = nc.dram_tensor("seT", [7, DIMS, N], bf, kind="ExternalInput")
    deT_d = nc.dram_tensor("deT", [7, DIMS, N], bf, kind="ExternalInput")
    wc_d = nc.dram_tensor("wc", [WREG.size], bf, kind="ExternalInput")
    bc_d = nc.dram_tensor("bc", [BREG.size], f32, kind="ExternalInput")
    out_d = nc.dram_tensor("out", [bl, OUT, N, 1], f32, kind="ExternalOutput")

    A_d = nc.dram_tensor("A_scr", [7, bl, N, N], bf)
    A2_d = nc.dram_tensor("A2_scr", [7, bl, N, N], bf)
    y_d = [nc.dram_tensor(f"y{l}", [bl, 4, CT_OUT[l], N], bf) for l in range(4)]
    skip_d = nc.dram_tensor("skip_scr", [bl, 304, N], bf)
    dbg_xg = nc.dram_tensor("dbg_xg", [4, 240, N], bf)
    dbg_tf = nc.dram_tensor("dbg_tf", [240, N], bf)
    dbg_tg = nc.dram_tensor("dbg_tg", [240, N], bf)
    dbg_x1 = nc.dram_tensor("dbg_x1", [240, N], bf)
    dbg_x2 = nc.dram_tensor("dbg_x2", [240, N], bf)
    dbg_xgT = nc.dram_tensor("dbg_xgT", [500, 240], bf)
    dbg_aux = nc.dram_tensor("dbg_aux", [3, 240, N], bf)
    stf_d = [nc.dram_tensor(f"stf{l}_{s}", [CT_OUT[l] * 2], f32)
             for l in range(4) for s in range(4)]
    stin_d = [nc.dram_tensor(f"stin{l}", [16, 8], f32) for l in range(4)]
    stout_d = [nc.dram_tensor(f"stout{l}", [16, 8], f32) for l in range(4)]

    def wslice(name):
        off, shape = WREG.off(name)
        n = int(np.prod(shape))
        ap = wc_d.ap()[off:off + n]
        if len(shape) == 2:
            ap = ap.rearrange("(p q) -> p q", q=shape[1])
        return ap

    def bslice(name):
        off, shape = BREG.off(name)
        n = int(np.prod(shape))
        ap = bc_d.ap()[off:off + n]
        if len(shape) == 2:
            ap = ap.rearrange("(p q) -> p q", q=shape[1])
        else:
            ap = ap.rearrange("(p q) -> p q", q=1)
        return ap

    eng_alt = [0]

    def copy_out(dst, src):
        """psum->sbuf copy alternating DVE/ACT."""
        eng_alt[0] ^= 1
        if eng_alt[0]:
            nc.vector.tensor_copy(dst, src)
        else:
            nc.scalar.copy(dst, src)

    with tile.TileContext(nc) as tc, ExitStack() as ctx:
        # ---- persistent pools ----
        glob = ctx.enter_context(tc.tile_pool(name="glob", bufs=1))
        ident = glob.tile([128, 128], bf, tag="ident", name="ident")
        masks.make_identity(nc, ident[:])
        ones = glob.tile([128, 1], bf, tag="ones", name="ones")
        nc.vector.memset(ones[:], 1.0)

        # =========================== Phase A ===========================
        with tc.tile_pool(name="pa_sb", bufs=1) as pa, \
             tc.tile_pool(name="pa_ps", bufs=1, space="PSUM") as pa_ps, \
             tc.tile_pool(name="pa_ps2", bufs=2, space="PSUM") as pa_ps2:
            for g in range(7):
                seT_t = pa.tile([DIMS, N], bf, tag="seT", name="seT")
                deT_t = pa.tile([DIMS, N], bf, tag="deT", name="deT")
                nc.sync.dma_start(seT_t[:], seT_d.ap()[g])
                nc.sync.dma_start(deT_t[:], deT_d.ap()[g])
                for a in range(bl):
                    t1_t = pa.tile([DIMS, DIMS], bf, tag="t1", name="t1")
                    nc.sync.dma_start(t1_t[:], t1_d.ap()[g, a])
                    p_adp = pa_ps.tile([DIMS, N], f32, tag="padp", name="padp")
                    nc.tensor.matmul(p_adp[:], t1_t[:], seT_t[:], start=True, stop=True)
                    adp2T = pa.tile([DIMS, N], bf, tag="adp2T", name="adp2T")
                    nc.scalar.copy(adp2T[:], p_adp[:])
                    # S rows (w on partitions) and S_T rows (v on partitions)
                    expS, expST, rcol = [], [], []
                    for vi, (vo, vw) in enumerate(V_TILES):
                        pS = pa_ps2.tile([125, N], f32, tag="pS", name="pS")
                        nc.tensor.matmul(pS[:vw], adp2T[:, vo:vo + vw], deT_t[:],
                                         start=True, stop=True)
                        rS = pa.tile([125, N], bf, tag=f"rS{vi}", name=f"rS{vi}")
                        nc.vector.tensor_scalar_max(rS[:vw], pS[:vw], 0.0)
                        eS = pa.tile([125, N], bf, tag=f"eS{vi}", name=f"eS{vi}")
                        rsum = pa.tile([125, 1], f32, tag=f"rsum{vi}", name=f"rsum{vi}")
                        nc.scalar.activation(eS[:vw], rS[:vw], AF.Exp, accum_out=rsum[:vw])
                        rc = pa.tile([125, 1], f32, tag=f"rc{vi}", name=f"rc{vi}")
                        nc.vector.reciprocal(rc[:vw], rsum[:vw])
                        expS.append(eS); rcol.append(rc)
                        pT = pa_ps2.tile([125, N], f32, tag="pT", name="pT")
                        nc.tensor.matmul(pT[:vw], deT_t[:, vo:vo + vw], adp2T[:],
                                         start=True, stop=True)
                        rT = pa.tile([125, N], bf, tag=f"rT{vi}", name=f"rT{vi}")
                        nc.vector.tensor_scalar_max(rT[:vw], pT[:vw], 0.0)
                        eT = pa.tile([125, N], bf, tag=f"eT{vi}", name=f"eT{vi}")
                        nc.scalar.activation(eT[:vw], rT[:vw], AF.Exp)
                        expST.append(eT)
                    # column sums of expST via ones-matmul
                    p_cs = pa_ps.tile([1, N], f32, tag="pcs", name="pcs")
                    for vi, (vo, vw) in enumerate(V_TILES):
                        nc.tensor.matmul(p_cs[:], ones[:vw], expST[vi][:vw],
                                         start=(vi == 0), stop=(vi == 3))
                    rrow = pa.tile([1, N], f32, tag="rrow", name="rrow")
                    nc.vector.reciprocal(rrow[:], p_cs[:])
                    rbc = pa.tile([128, N], f32, tag="rbc", name="rbc")
                    nc.gpsimd.partition_broadcast(rbc[:], rrow[:], channels=128)
                    # normalize: A rows (w), A_T columns (w)
                    At_t, A_t = [], []
                    for vi, (vo, vw) in enumerate(V_TILES):
                        Ar = pa.tile([125, N], bf, tag=f"Ar{vi}", name=f"Ar{vi}")
                        nc.vector.tensor_scalar_mul(Ar[:vw], expS[vi][:vw], rcol[vi][:vw])
                        A_t.append(Ar)
                        Atr = pa.tile([125, N], bf, tag=f"Atr{vi}", name=f"Atr{vi}")
                        nc.vector.tensor_mul(Atr[:vw], expST[vi][:vw], rbc[:vw])
                        At_t.append(Atr)
                        nc.sync.dma_start(A_d.ap()[g, a, vo:vo + vw, :], Atr[:vw])
                    # A2_T = A.T @ A_T  (lhsT = A tiles)
                    for mi, (mo, mw) in enumerate(V_TILES):
                        pA2 = pa_ps2.tile([125, N], f32, tag="pA2", name="pA2")
                        for ki, (ko, kw) in enumerate(V_TILES):
                            nc.tensor.matmul(pA2[:mw], A_t[ki][:kw, mo:mo + mw],
                                             At_t[ki][:kw],
                                             start=(ki == 0), stop=(ki == 3))
                        A2s = pa.tile([125, N], bf, tag="A2s", name="A2s")
                        copy_out(A2s[:mw], pA2[:mw])
                        nc.sync.dma_start(A2_d.ap()[g, a, mo:mo + mw, :], A2s[:mw])

        # =========================== Layers ===========================
        wpool = ctx.enter_context(tc.tile_pool(name="wpool", bufs=1))
        dyn = ctx.enter_context(tc.tile_pool(name="dyn", bufs=1))
        apool = ctx.enter_context(tc.tile_pool(name="apool", bufs=2))
        act = ctx.enter_context(tc.tile_pool(name="act", bufs=1))
        act2 = ctx.enter_context(tc.tile_pool(name="act2", bufs=2))
        stat = ctx.enter_context(tc.tile_pool(name="stat", bufs=1))
        ps_y = ctx.enter_context(tc.tile_pool(name="ps_y", bufs=2, space="PSUM"))
        ps_x = ctx.enter_context(tc.tile_pool(name="ps_x", bufs=2, space="PSUM"))
        ps_tr = ctx.enter_context(tc.tile_pool(name="ps_tr", bufs=1, space="PSUM"))
        ps_s = ctx.enter_context(tc.tile_pool(name="ps_s", bufs=1, space="PSUM"))
        dr = ctx.enter_context(tc.tile_pool(name="dr", bufs=2, space="DRAM"))

        # bn fold state (set at end of each layer for the next one)
        fold = {}

        def load_w(name, tag=None):
            """Load a 2-D wconst matrix as a list of <=128-partition sbuf tiles."""
            off, shape = WREG.off(name)
            rows, cols = shape
            out = []
            for i, (o, w) in enumerate(pt_tiles(rows)):
                t = wpool.tile([w, cols], bf, tag=tag or f"{name}_{i}")
                nc.sync.dma_start(t[:], wslice(name)[o:o + w, :])
                out.append((t, o, w))
            return out

        def load_b(name, tag=None):
            off, shape = BREG.off(name)
            rows = shape[0]
            out = []
            for i, (o, w) in enumerate(pt_tiles(rows)):
                t = wpool.tile([w, 1], f32, tag=tag or f"{name}_b{i}")
                nc.sync.dma_start(t[:], bslice(name)[o:o + w, :])
                out.append((t, o, w))
            return out

        for l in range(4):
            ct_in, ct_out = CT_IN[l], CT_OUT[l]
            in_tiles = pt_tiles(ct_in)
            out_tiles = pt_tiles(ct_out)
            Tn = T_OUTS[l]

            # ---- layer constants ----
            G_t = {}
            for g in range(7):
                for m in range(3):
                    G_t[(g, m)] = load_w(f"G_{g}_{l}_{m}")
            Sk_t = load_w(f"Sk_{l}")
            gcbp_t = load_b(f"gcbp_{l}")
            gcba_t = {j: load_b(f"gcba_{l}_{j}") for j in (1, 2, 3)}

            # ---- per-stream weights with bn folding ----
            Wf_s, Wg_s, bf_sl, bg_sl, Rs_s, bres_s = {}, {}, {}, {}, {}, {}
            if l == 0:
                for s in range(4):
                    Wf_s[s] = load_w(f"Wf_{l}_{s}")
                    Wg_s[s] = load_w(f"Wg_{l}_{s}")
                    bf_sl[s] = load_b(f"bf_{l}_{s}")
                    bg_sl[s] = load_b(f"bg_{l}_{s}")
                    Rs_s[s] = load_w(f"Rs0_{s}")
                    bres_s[s] = load_b(f"bres0_{s}")
            else:
                aexp, bexp, bexp_bf = fold["aexp"], fold["bexp"], fold["bexp_bf"]
                Rbase = load_w(f"Rsel_{l}")
                for s in range(4):
                    Wfb = load_w(f"Wf_{l}_{s}")
                    Wgb = load_w(f"Wg_{l}_{s}")
                    bfc = load_b(f"bf_{l}_{s}")
                    bgc = load_b(f"bg_{l}_{s}")
                    Wf_s[s], Wg_s[s], Rs_s[s] = [], [], []
                    for ki, (ko, kw) in enumerate(in_tiles):
                        wt = dyn.tile([kw, ct_out], bf, tag=f"Wfd{s}_{ki}_{l%2}", name=f"Wfd{s}_{ki}_{l%2}")
                        nc.vector.tensor_scalar_mul(wt[:], Wfb[ki][0][:], aexp[s][ki][:kw])
                        Wf_s[s].append((wt, ko, kw))
                        wt2 = dyn.tile([kw, ct_out], bf, tag=f"Wgd{s}_{ki}_{l%2}", name=f"Wgd{s}_{ki}_{l%2}")
                        nc.vector.tensor_scalar_mul(wt2[:], Wgb[ki][0][:], aexp[s][ki][:kw])
                        Wg_s[s].append((wt2, ko, kw))
                        rt = dyn.tile([kw, ct_out], bf, tag=f"Rsd{s}_{ki}_{l%2}", name=f"Rsd{s}_{ki}_{l%2}")
                        nc.vector.tensor_scalar_mul(rt[:], Rbase[ki][0][:], aexp[s][ki][:kw])
                        Rs_s[s].append((rt, ko, kw))
                    # bias folds: bf + Wb^T @ bexp
                    bf_sl[s], bg_sl[s] = [], []
                    for mi, (mo, mw) in enumerate(out_tiles):
                        pb = ps_s.tile([128, 1], f32, tag="psmall", name="psmall")
                        for ki, (ko, kw) in enumerate(in_tiles):
                            nc.tensor.matmul(pb[:mw], Wfb[ki][0][:, mo:mo + mw],
                                             bexp_bf[s][ki][:kw],
                                             start=(ki == 0), stop=(ki == len(in_tiles) - 1))
                        bt = dyn.tile([mw, 1], f32, tag=f"bfd{s}_{mi}_{l%2}", name=f"bfd{s}_{mi}_{l%2}")
                        nc.vector.tensor_add(bt[:], pb[:mw], bfc[mi][0][:])
                        bf_sl[s].append((bt, mo, mw))
                        pb2 = ps_s.tile([128, 1], f32, tag="psmall", name="psmall")
                        for ki, (ko, kw) in enumerate(in_tiles):
                            nc.tensor.matmul(pb2[:mw], Wgb[ki][0][:, mo:mo + mw],
                                             bexp_bf[s][ki][:kw],
                                             start=(ki == 0), stop=(ki == len(in_tiles) - 1))
                        bt2 = dyn.tile([mw, 1], f32, tag=f"bgd{s}_{mi}_{l%2}", name=f"bgd{s}_{mi}_{l%2}")
                        nc.vector.tensor_add(bt2[:], pb2[:mw], bgc[mi][0][:])
                        bg_sl[s].append((bt2, mo, mw))
                    # residual shift at T_out: bres = Exp_l @ bnB
                    bres_s[s] = []
                    Expl = fold["Expl"]
                    for mi, (mo, mw) in enumerate(out_tiles):
                        pe = ps_s.tile([128, 1], f32, tag="psmall", name="psmall")
                        nc.tensor.matmul(pe[:mw], Expl[0][0][:, mo:mo + mw],
                                         fold["bnB_bf"][:, s:s + 1], start=True, stop=True)
                        et = dyn.tile([mw, 1], f32, tag=f"bres{s}_{mi}_{l%2}", name=f"bres{s}_{mi}_{l%2}")
                        nc.vector.tensor_copy(et[:], pe[:mw])
                        bres_s[s].append((et, mo, mw))

            # primary combined bias: gcbp + bres
            by_p = []
            for mi, (mo, mw) in enumerate(out_tiles):
                t = dyn.tile([mw, 1], f32, tag=f"byp_{mi}_{l%2}", name=f"byp_{mi}_{l%2}")
                nc.vector.tensor_add(t[:], gcbp_t[mi][0][:], bres_s[0][mi][0][:])
                by_p.append((t, mo, mw))

            # stats accumulators [ct_out, 2, 8]
            st_s = {}
            for s in range(4):
                st_s[s] = [stat.tile([w, 2 * bl], f32, tag=f"st{s}_{i}_{l%2}", name=f"st{s}_{i}_{l%2}")
                           for i, (o, w) in enumerate(out_tiles)]
            sq_dump = act.tile([128, N], f32, tag="sqdump", name="sqdump")

            # ---------------- batch loop ----------------
            for b in range(bl):
                # y_in tiles
                y_in = {}
                for s in range(4):
                    y_in[s] = []
                    for ki, (ko, kw) in enumerate(in_tiles):
                        t = act2.tile([kw, N], bf, tag=f"yin{s}_{ki}", name=f"yin{s}_{ki}")
                        if l == 0:
                            nc.sync.dma_start(t[:], xo_d.ap()[b, s])
                        else:
                            nc.sync.dma_start(t[:], y_d[l - 1].ap()[b, s, ko:ko + kw, :])
                        y_in[s].append((t, ko, kw))

                # dilconv + gating
                xg, xgT = {}, {}
                for s in range(4):
                    xg[s] = []
                    for mi, (mo, mw) in enumerate(out_tiles):
                        pf = ps_x.tile([128, N], f32, tag="pdil", name="pdil")
                        for ki, (ko, kw) in enumerate(in_tiles):
                            nc.tensor.matmul(pf[:mw], Wf_s[s][ki][0][:, mo:mo + mw],
                                             y_in[s][ki][0][:],
                                             start=(ki == 0), stop=(ki == len(in_tiles) - 1))
                        tf = act.tile([mw, N], bf, tag=f"tf{s}_{mi}", name=f"tf{s}_{mi}")
                        nc.scalar.activation(tf[:], pf[:mw], AF.Tanh, bias=bf_sl[s][mi][0][:])
                        pg = ps_x.tile([128, N], f32, tag="pdil", name="pdil")
                        for ki, (ko, kw) in enumerate(in_tiles):
                            nc.tensor.matmul(pg[:mw], Wg_s[s][ki][0][:, mo:mo + mw],
                                             y_in[s][ki][0][:],
                                             start=(ki == 0), stop=(ki == len(in_tiles) - 1))
                        tg = act.tile([mw, N], bf, tag=f"tg{s}_{mi}", name=f"tg{s}_{mi}")
                        nc.scalar.activation(tg[:], pg[:mw], AF.Sigmoid, bias=bg_sl[s][mi][0][:])
                        xt = act.tile([mw, N], bf, tag=f"xg{s}_{mi}", name=f"xg{s}_{mi}")
                        nc.vector.tensor_mul(xt[:], tf[:], tg[:])
                        xg[s].append((xt, mo, mw))
                        if l == 0 and b == 0:
                            nc.sync.dma_start(dbg_xg.ap()[s, mo:mo + mw, :], xt[:])
                            if s == 0:
                                nc.sync.dma_start(dbg_tf.ap()[mo:mo + mw, :], tf[:])
                                nc.sync.dma_start(dbg_tg.ap()[mo:mo + mw, :], tg[:])
                    # transpose xg -> xgT [v, ct_out]
                    xgT[s] = []
                    for vi, (vo, vw) in enumerate(V_TILES):
                        pt = ps_tr.tile([125, 256], bf, tag="ptr", name="ptr")
                        for mi, (mo, mw) in enumerate(out_tiles):
                            nc.tensor.transpose(pt[:vw, mo:mo + mw],
                                                xg[s][mi][0][:, vo:vo + vw],
                                                ident[:mw, :mw])
                        xt = act.tile([125, ct_out], bf, tag=f"xgT{s}_{vi}", name=f"xgT{s}_{vi}")
                        copy_out(xt[:vw], pt[:vw, :ct_out])
                        xgT[s].append(xt)
                        if l == 0 and b == 0 and s == 0:
                            nc.sync.dma_start(dbg_xgT.ap()[vo:vo + vw, :], xt[:vw])

                # skip (primary stream)
                psk = ps_s.tile([SCT[0], N], f32, tag="psmall", name="psmall")
                for ki, (ko, kw) in enumerate(out_tiles):
                    nc.tensor.matmul(psk[:SCT[l]], Sk_t[ki][0][:, :],
                                     xg[0][ki][0][:],
                                     start=(ki == 0), stop=(ki == len(out_tiles) - 1))
                sk_sb = act.tile([SCT[0], N], bf, tag="sk_sb", name="sk_sb")
                copy_out(sk_sb[:SCT[l]], psk[:SCT[l]])
                nc.sync.dma_start(
                    skip_d.ap()[b, SKIP_OFF[l]:SKIP_OFF[l] + SCT[l], :], sk_sb[:SCT[l]])

                def load_A(g):
                    At, A2t = [], []
                    for vi, (vo, vw) in enumerate(V_TILES):
                        t = apool.tile([125, N], bf, tag=f"A_{vi}", name=f"A_{vi}")
                        nc.sync.dma_start(t[:vw], A_d.ap()[g, b, vo:vo + vw, :])
                        At.append(t)
                        t2 = apool.tile([125, N], bf, tag=f"A2_{vi}", name=f"A2_{vi}")
                        nc.sync.dma_start(t2[:vw], A2_d.ap()[g, b, vo:vo + vw, :])
                        A2t.append(t2)
                    return At, A2t

                def nconv_pair(srcT, At, A2t):
                    """x1T/x2T [ct_out, N] sbuf tiles from transposed source."""
                    x1, x2 = [], []
                    for mi, (mo, mw) in enumerate(out_tiles):
                        p1 = ps_x.tile([128, N], f32, tag="px", name="px")
                        p2 = ps_x.tile([128, N], f32, tag="px", name="px")
                        for vi, (vo, vw) in enumerate(V_TILES):
                            nc.tensor.matmul(p1[:mw], srcT[vi][:vw, mo:mo + mw],
                                             At[vi][:vw], start=(vi == 0), stop=(vi == 3))
                            nc.tensor.matmul(p2[:mw], srcT[vi][:vw, mo:mo + mw],
                                             A2t[vi][:vw], start=(vi == 0), stop=(vi == 3))
                        s1 = act.tile([128, N], bf, tag=f"x1T_{mi}", name=f"x1T_{mi}")
                        s2 = act.tile([128, N], bf, tag=f"x2T_{mi}", name=f"x2T_{mi}")
                        copy_out(s1[:mw], p1[:mw])
                        copy_out(s2[:mw], p2[:mw])
                        x1.append((s1, mo, mw))
                        x2.append((s2, mo, mw))
                    return x1, x2

                def gcn_mms(py, g, src_tiles, x1, x2, mi, mo, mw, start, close=False):
                    first = start
                    nk = len(out_tiles)
                    for ki, (ko, kw) in enumerate(out_tiles):
                        last = close and ki == nk - 1
                        nc.tensor.matmul(py[:mw], G_t[(g, 0)][ki][0][:, mo:mo + mw],
                                         src_tiles[ki][0][:kw], start=first, stop=False)
                        first = False
                        nc.tensor.matmul(py[:mw], G_t[(g, 1)][ki][0][:, mo:mo + mw],
                                         x1[ki][0][:kw], start=False, stop=False)
                        nc.tensor.matmul(py[:mw], G_t[(g, 2)][ki][0][:, mo:mo + mw],
                                         x2[ki][0][:kw], start=False, stop=last)
                    return first

                # ---- aux streams (groups 1..3) ----
                aux_out, auxT = {}, {}
                for j in (1, 2, 3):
                    At, A2t = load_A(j)
                    x1, x2 = nconv_pair(xgT[j], At, A2t)
                    aux_out[j] = []
                    py_l = []
                    for mi, (mo, mw) in enumerate(out_tiles):
                        py = ps_y.tile([128, N], f32, tag="py", name="py")
                        gcn_mms(py, j, xg[j], x1, x2, mi, mo, mw, True, close=True)
                        # close group for the aux_out snapshot
                        ao = act.tile([128, N], bf, tag=f"aux{j}_{mi}", name=f"aux{j}_{mi}")
                        nc.scalar.activation(ao[:mw], py[:mw], AF.Identity,
                                             bias=gcba_t[j][mi][0][:])
                        aux_out[j].append((ao, mo, mw))
                        if l == 0 and b == 0:
                            nc.sync.dma_start(dbg_aux.ap()[j - 1, mo:mo + mw, :], ao[:mw])
                        py_l.append(py)
                    # transpose aux_out
                    auxT[j] = []
                    for vi, (vo, vw) in enumerate(V_TILES):
                        ptr = ps_tr.tile([125, 256], bf, tag="ptr", name="ptr")
                        for mi, (mo, mw) in enumerate(out_tiles):
                            nc.tensor.transpose(ptr[:vw, mo:mo + mw],
                                                aux_out[j][mi][0][:mw, vo:vo + vw],
                                                ident[:mw, :mw])
                        xt = act.tile([125, ct_out], bf, tag=f"auxT{j}_{vi}", name=f"auxT{j}_{vi}")
                        copy_out(xt[:vw], ptr[:vw, :ct_out])
                        auxT[j].append(xt)
                    # residual into the same psum, then final y copy + stats
                    for mi, (mo, mw) in enumerate(out_tiles):
                        py = py_l[mi]
                        for ki, (ko, kw) in enumerate(in_tiles):
                            nc.tensor.matmul(py[:mw], Rs_s[j][ki][0][:, mo:mo + mw],
                                             y_in[j][ki][0][:], start=False,
                                             stop=(ki == len(in_tiles) - 1),
                                             skip_group_check=True)
                        yo = act.tile([128, N], bf, tag=f"yo{j}_{mi}", name=f"yo{j}_{mi}")
                        nc.scalar.activation(yo[:mw], py[:mw], AF.Identity,
                                             bias=bres_s[j][mi][0][:],
                                             accum_out=st_s[j][mi][:mw, b:b + 1])
                        nc.scalar.activation(
                            sq_dump[:mw], yo[:mw], AF.Square,
                            accum_out=st_s[j][mi][:mw, bl + b:bl + b + 1])
                        nc.sync.dma_start(y_d[l].ap()[b, j, mo:mo + mw, :], yo[:mw])

                # ---- primary stream ----
                At0, A2t0 = load_A(0)
                x1p, x2p = nconv_pair(xgT[0], At0, A2t0)
                if l == 0 and b == 0:
                    for mi, (mo, mw) in enumerate(out_tiles):
                        nc.sync.dma_start(dbg_x1.ap()[mo:mo + mw, :], x1p[mi][0][:mw])
                        nc.sync.dma_start(dbg_x2.ap()[mo:mo + mw, :], x2p[mi][0][:mw])
                py_p = []
                for mi, (mo, mw) in enumerate(out_tiles):
                    py = ps_y.tile([128, N], f32, tag="py", name="py")
                    # residual first (opens the accumulation group)
                    for ki, (ko, kw) in enumerate(in_tiles):
                        nc.tensor.matmul(py[:mw], Rs_s[0][ki][0][:, mo:mo + mw],
                                         y_in[0][ki][0][:], start=(ki == 0), stop=False)
                    gcn_mms(py, 0, xg[0], x1p, x2p, mi, mo, mw, False)
                    py_p.append(py)
                for j in (1, 2, 3):
                    g = 3 + j
                    At, A2t = load_A(g)
                    x1, x2 = nconv_pair(auxT[j], At, A2t)
                    for mi, (mo, mw) in enumerate(out_tiles):
                        last = (j == 3)
                        for ki, (ko, kw) in enumerate(out_tiles):
                            nc.tensor.matmul(py_p[mi][:mw], G_t[(g, 0)][ki][0][:, mo:mo + mw],
                                             aux_out[j][ki][0][:kw], start=False, stop=False)
                            nc.tensor.matmul(py_p[mi][:mw], G_t[(g, 1)][ki][0][:, mo:mo + mw],
                                             x1[ki][0][:kw], start=False, stop=False)
                            nc.tensor.matmul(py_p[mi][:mw], G_t[(g, 2)][ki][0][:, mo:mo + mw],
                                             x2[ki][0][:kw], start=False,
                                             stop=(last and ki == len(out_tiles) - 1))
                for mi, (mo, mw) in enumerate(out_tiles):
                    yo = act.tile([128, N], bf, tag=f"yo0_{mi}", name=f"yo0_{mi}")
                    nc.scalar.activation(yo[:mw], py_p[mi][:mw], AF.Identity,
                                         bias=by_p[mi][0][:],
                                         accum_out=st_s[0][mi][:mw, b:b + 1])
                    nc.scalar.activation(
                        sq_dump[:mw], yo[:mw], AF.Square,
                        accum_out=st_s[0][mi][:mw, bl + b:bl + b + 1])
                    nc.sync.dma_start(y_d[l].ap()[b, 0, mo:mo + mw, :], yo[:mw])

            # ---------------- stats fold + collective + bn ----------------
            statsall = stat.tile([16, 8], f32, tag=f"sall_{l%2}", name=f"sall_{l%2}")
            for s in range(4):
                # reduce over b (free dim), then t via DRAM roundtrip
                red = stat.tile([128, 2], f32, tag="redtmp", name="redtmp")
                for i, (o, w) in enumerate(out_tiles):
                    nc.vector.tensor_reduce(
                        red[:w], st_s[s][i][:w].rearrange("p (q b) -> p q b", b=bl),
                        axis=mybir.AxisListType.X, op=ALU.add)
                    nc.sync.dma_start(
                        stf_d[l * 4 + s].ap()[o * 2:(o + w) * 2].rearrange("(p q) -> p q", q=2),
                        red[:w])
                back = stat.tile([16, Tn, 2], f32, tag="backtmp", name="backtmp")
                nc.sync.dma_start(
                    back[:], stf_d[l * 4 + s].ap().rearrange("(c t q) -> c t q", c=16, q=2))
                nc.vector.tensor_reduce(statsall[:, 2 * s:2 * s + 2],
                                        back[:].rearrange("c t q -> c q t"),
                                        axis=mybir.AxisListType.X, op=ALU.add)
            # collective
            nc.sync.dma_start(stin_d[l].ap(), statsall[:])
            nc.gpsimd.collective_compute(
                "AllReduce", ALU.add, replica_groups=[list(range(ncores))],
                ins=[stin_d[l].ap()], outs=[stout_d[l].ap()])
            stg = stat.tile([16, 8], f32, tag=f"stg_{l%2}", name=f"stg_{l%2}")
            nc.sync.dma_start(stg[:], stout_d[l].ap())

            Nf = float(B * N * Tn)
            stg3 = stg[:].rearrange("c (s q) -> c q s", q=2)
            mean = stat.tile([16, 4], f32, tag="mean", name="mean")
            nc.vector.tensor_scalar_mul(mean[:], stg3[:, 0:1, :], 1.0 / Nf)
            msq = stat.tile([16, 4], f32, tag="msq", name="msq")
            nc.vector.tensor_scalar_mul(msq[:], stg3[:, 1:2, :], 1.0 / Nf)
            var = stat.tile([16, 4], f32, tag="var", name="var")
            nc.vector.scalar_tensor_tensor(var[:], mean[:], -1.0, mean[:],
                                           op0=ALU.mult, op1=ALU.mult)
            nc.vector.tensor_add(var[:], var[:], msq[:])
            nc.vector.tensor_scalar_add(var[:], var[:], EPS)
            lnv = stat.tile([16, 4], f32, tag="lnv", name="lnv")
            nc.scalar.activation(lnv[:], var[:], AF.Ln)
            nc.vector.tensor_scalar_mul(lnv[:], lnv[:], -0.5)
            rsq = stat.tile([16, 4], f32, tag="rsq", name="rsq")
            nc.scalar.activation(rsq[:], lnv[:], AF.Exp)
            bng_t = stat.tile([16, 4], f32, tag="bng", name="bng")
            bnb_t = stat.tile([16, 4], f32, tag="bnb", name="bnb")
            nc.sync.dma_start(bng_t[:], bslice(f"bng_{l}"))
            nc.sync.dma_start(bnb_t[:], bslice(f"bnb_{l}"))
            bnA = stat.tile([16, 4], f32, tag=f"bnA_{l%2}", name=f"bnA_{l%2}")
            nc.vector.tensor_mul(bnA[:], rsq[:], bng_t[:])
            bnB = stat.tile([16, 4], f32, tag=f"bnB_{l%2}", name=f"bnB_{l%2}")
            nc.vector.scalar_tensor_tensor(bnB[:], mean[:], -1.0, bnA[:],
                                           op0=ALU.mult, op1=ALU.mult)
            nc.vector.tensor_add(bnB[:], bnB[:], bnb_t[:])
            bnA_bf = stat.tile([16, 4], bf, tag=f"bnAbf_{l%2}", name=f"bnAbf_{l%2}")
            nc.vector.tensor_copy(bnA_bf[:], bnA[:])
            bnB_bf = stat.tile([16, 4], bf, tag=f"bnBbf_{l%2}", name=f"bnBbf_{l%2}")
            nc.vector.tensor_copy(bnB_bf[:], bnB[:])

            # expansions for next layer
            if l < 3:
                nin_tiles = pt_tiles(CT_IN[l + 1])
                Expl = load_w(f"Exp_{l}")          # [16, 16*T_OUTS[l]] = ct_in of l+1
                Expl_next = load_w(f"Exp_{l + 1}")
                aexp, bexp, bexp_bf = {}, {}, {}
                for s in range(4):
                    aexp[s], bexp[s], bexp_bf[s] = [], [], []
                    for ki, (ko, kw) in enumerate(nin_tiles):
                        pe = ps_s.tile([128, 1], f32, tag="psmall", name="psmall")
                        nc.tensor.matmul(pe[:kw], Expl[0][0][:, ko:ko + kw],
                                         bnA_bf[:, s:s + 1], start=True, stop=True)
                        at = dyn.tile([kw, 1], f32, tag=f"aexp{s}_{ki}_{l%2}", name=f"aexp{s}_{ki}_{l%2}")
                        nc.vector.tensor_copy(at[:], pe[:kw])
                        aexp[s].append(at)
                        pe2 = ps_s.tile([128, 1], f32, tag="psmall", name="psmall")
                        nc.tensor.matmul(pe2[:kw], Expl[0][0][:, ko:ko + kw],
                                         bnB_bf[:, s:s + 1], start=True, stop=True)
                        bt = dyn.tile([kw, 1], f32, tag=f"bexp{s}_{ki}_{l%2}", name=f"bexp{s}_{ki}_{l%2}")
                        nc.vector.tensor_copy(bt[:], pe2[:kw])
                        bexp[s].append(bt)
                        btb = dyn.tile([kw, 1], bf, tag=f"bexpbf{s}_{ki}_{l%2}", name=f"bexpbf{s}_{ki}_{l%2}")
                        nc.vector.tensor_copy(btb[:], pe2[:kw])
                        bexp_bf[s].append(btb)
                fold = {"aexp": aexp, "bexp": bexp, "bexp_bf": bexp_bf,
                        "Expl": Expl_next, "bnB_bf": bnB_bf}

        # =========================== Head ===========================
        e1 = load_w("end1_lhsT")
        e2 = load_w("end2_lhsT")
        skb_t = load_b("skb")
        e1b = load_b("end1_b")
        e2b = load_b("end2_b")
        for b in range(bl):
            hs = []
            for ki, (ko, kw) in enumerate(pt_tiles(304)):
                t = act.tile([kw, N], bf, tag=f"sk_in{ki}", name=f"sk_in{ki}")
                nc.sync.dma_start(t[:], skip_d.ap()[b, ko:ko + kw, :])
                h = act.tile([kw, N], bf, tag=f"sk_r{ki}", name=f"sk_r{ki}")
                nc.scalar.activation(h[:], t[:], mybir.ActivationFunctionType.Relu,
                                     bias=skb_t[ki][0][:])
                hs.append((h, ko, kw))
            ph = ps_s.tile([EC, N], f32, tag="psmall", name="psmall")
            for ki, (ko, kw) in enumerate(pt_tiles(304)):
                nc.tensor.matmul(ph[:], e1[ki][0][:, :], hs[ki][0][:],
                                 start=(ki == 0), stop=(ki == 2))
            h2 = act.tile([EC, N], bf, tag="h2", name="h2")
            nc.scalar.activation(h2[:], ph[:], mybir.ActivationFunctionType.Relu,
                                 bias=e1b[0][0][:])
            po = ps_s.tile([OUT, N], f32, tag="psmall", name="psmall")
            nc.tensor.matmul(po[:], e2[0][0][:, :], h2[:], start=True, stop=True)
            ob = act.tile([OUT, N], f32, tag="ob", name="ob")
            nc.scalar.activation(ob[:], po[:], mybir.ActivationFunctionType.Identity,
                                 bias=e2b[0][0][:])
            nc.sync.dma_start(out_d.ap()[b].rearrange("o n q -> o (n q)"), ob[:])

    nc.compile()
    return nc


def get_program(bl=BL, ncores=NCORES):
    key = (bl, ncores)
    if key not in _NC_CACHE:
        _NC_CACHE[key] = build_program(bl, ncores)
    return _NC_CACHE[key]


def kernel(**inputs):
    from concourse.bass_utils import run_bass_kernel_spmd

    wc, bc = host_constants(inputs)
    xo, t1, seT, deT = host_per_core(inputs)
    nc = get_program()
    in_maps = []
    for c in range(NCORES):
        sl = slice(c * BL, (c + 1) * BL)
        in_maps.append({
            "xo": np.ascontiguousarray(xo[sl]),
            "t1": np.ascontiguousarray(t1[:, sl]),
            "seT": seT, "deT": deT, "wc": wc, "bc": bc,
        })
    res = run_bass_kernel_spmd(nc, in_maps, list(range(NCORES)))
    out = np.concatenate([r["out"] for r in res.results], axis=0)
    return out.astype(np.float32)


if __name__ == "__main__":
    import reference as R
    inputs = R.setup_inputs()
    got = kernel(**inputs)
    exp = np.asarray(R.reference(**inputs))
    err = np.abs(got - exp)
    print("rel err:", err.max() / np.abs(exp).max())

